# revision 43
# baseline (speedup 1.0000x reference)
"""Trainium2 Bass kernel for fused attention + LayerNorm + projection.

Computation (per reference):
    q = input1 @ Wq + bq                       [8192, 32]
    k = input2 @ Wk + bk                       [8192, 32]
    v = input2 @ Wv + bv                       [8192, 32]
    P = softmax(q @ k.T, axis=-1)              [8192, 8192]
    fused = P @ v                              [8192, 32]
    out = LayerNorm(fused) * gamma + beta @ Wo + bo   [8192, 128]

Sharding: data-parallel over rows of input1 (1024 rows per core, 8 cores);
input2 and weights replicated.

Algebraic simplifications (host + device):
  - softmax normalization (and max-subtraction) is skipped: LayerNorm is
    invariant to a positive per-row scale, so exp(s) @ v is normalized for
    free by LN (eps term differs by ~1e-3 relative — validated vs reference).
  - gamma is folded into Wo (diag(gamma) @ Wo) on the host; the constant
    output row beta @ Wo + bo is added at the end.  The per-row 1/std of
    LayerNorm commutes with the projection, so it is applied to the
    *projected* row (out = rstd * (cent @ gWo) + bias) — this takes the
    ACT-table-loading rstd computation off the critical path of phase B.

Engine plan (all matmuls stream at 1 cycle/column):
  - The whole q/k/v/scores/AV matmul pipeline runs in bf16: DVE rounds
    f32 DMA data to bf16, PE transposes bf16 (1 cyc/row vs 2-4 for f32),
    k^T is one group-wide [32,512] matmul per 512 kv rows.
  - AV runs in natural orientation: out[m-block 128, 32] accumulates in
    PSUM over all 64 kv chunks with lhsT = exp-scores [128 n, 128 m] and
    rhs = v [128 n, 32]: 32-cycle matmuls at full 128-partition
    contraction (16k PE cycles vs 65k transposed), and `fused` lands
    natural so LayerNorm needs no transposes.  The PSUM accumulator uses
    exactly ONE start flag (hardware marks the whole 2KB zero-region
    pending-zero on start — a second start would wipe other blocks).
  - exp runs on ACT straight out of PSUM into bf16 SBUF tiles; at
    8.4M elements/core ACT is the pacing engine (~71 us busy), PE
    (~45 us) and DVE (~40 us) hide under it.
  - m-passes are outer (p=0 rows 0:512 with k/v prep, then p=1): phase-B
    stats/centering for the first half overlap the second attention pass.
  - All small constants ship as ONE packed [128, 611] DMA + one bulk
    bf16 conversion; input1 loads as two half DMAs so the q-prep chain
    starts sooner; output stages into one [128, 8, 128] tile and leaves
    as ONE DMA.
"""

import os
import sys

import numpy as np

N1 = 8192
N2 = 8192
DIN = 128
D = 32
DOUT = 128
NCORES = 8
MSH = N1 // NCORES          # rows per core
NCH = N2 // 128             # 64 in2 chunks
NG = NCH // 4               # 16 groups of 4 chunks
NB = MSH // 128             # 8 output m-blocks
LN_EPS = 1e-5

# packed-constant column layouts: bf16 block (matmul operands, shipped
# pre-converted) and f32 block (bias/eps scalars)
CB_ID = 0
CB_WQ = 128
CB_WK = 160
CB_WV = 192
CB_WOG = 224
CB_W = 352
CF_BVB = 0
CF_BIAS = 128
CF_BQ = 256
CF_BK = 257
CF_EPS = 258
CF_W = 259

_CACHE = {}


def _import_concourse():
    try:
        import concourse.bass  # noqa: F401
    except ImportError:
        for p in ("/opt/trn_rl_repo", os.path.expanduser("~/.axon_site/_ro/trn_rl_repo")):
            if os.path.isdir(p) and p not in sys.path:
                sys.path.insert(0, p)


def build(reps=1):
    """Build (and cache) the compiled single-core SPMD Bass program.

    reps > 1 repeats the whole computation serially (for wall-clock slope
    timing); the output is rewritten identically each rep.
    """
    key = ("nc", reps)
    if key in _CACHE:
        return _CACHE[key]
    _import_concourse()
    import concourse.bacc as bacc
    import concourse.tile as tile
    from concourse import mybir

    f32 = mybir.dt.float32
    bf16 = mybir.dt.bfloat16
    AF = mybir.ActivationFunctionType
    OP = mybir.AluOpType

    nc = bacc.Bacc(None, target_bir_lowering=False, debug=False)

    x1 = nc.dram_tensor("x1", [MSH, DIN], f32, kind="ExternalInput")
    x2 = nc.dram_tensor("x2", [N2, DIN], f32, kind="ExternalInput")
    cb_d = nc.dram_tensor("cpackb", [128, CB_W], bf16, kind="ExternalInput")
    cf_d = nc.dram_tensor("cpackf", [128, CF_W], f32, kind="ExternalInput")
    out_d = nc.dram_tensor("out", [MSH, DOUT], f32, kind="ExternalOutput")
    debug = bool(os.environ.get("KDEBUG"))
    if debug:
        dbg_qt = nc.dram_tensor("dbg_qt", [D, MSH], bf16, kind="ExternalOutput")
        dbg_k = nc.dram_tensor("dbg_k", [D, NCH * 128], bf16, kind="ExternalOutput")
        dbg_v = nc.dram_tensor("dbg_v", [128, NCH * D], bf16, kind="ExternalOutput")
        dbg_av = nc.dram_tensor("dbg_av", [128, NB * D], f32, kind="ExternalOutput")

    from contextlib import ExitStack

    with tile.TileContext(nc) as tc, ExitStack() as outer:
        consts = outer.enter_context(tc.tile_pool(name="consts", bufs=1))
        persist = outer.enter_context(tc.tile_pool(name="persist", bufs=1))
        av_pool = outer.enter_context(
            tc.tile_pool(name="av_ps", bufs=1, space="PSUM")
        )
        # phase-B SBUF state lives at rep scope so the first-half stats can
        # overlap the second attention pass inside the phase-A pool scope
        pbsb = outer.enter_context(tc.tile_pool(name="pbsb", bufs=1))

        kstack = persist.tile([D, NCH * 128], bf16)   # kT chunk c at cols 128c
        vstack = persist.tile([128, NCH * D], bf16)   # v chunk c at cols 32c
        qt = persist.tile([D, MSH], bf16)             # qT, single band
        x1t = persist.tile([128, MSH], bf16)          # input1 shard transposed

        for _rep in range(reps):
          with (
            tc.tile_pool(name="x2load", bufs=3) as x2load,
            tc.tile_pool(name="xb", bufs=2) as xbp,
            tc.tile_pool(name="x2t_sb", bufs=2) as x2tp,
            tc.tile_pool(name="tp_ps", bufs=2, space="PSUM") as tp_ps,
            tc.tile_pool(name="kv_ps", bufs=1, space="PSUM") as kv_ps,
            tc.tile_pool(name="sc_ps", bufs=2, space="PSUM") as sc_ps,
            tc.tile_pool(name="pt", bufs=4) as ptp,
          ):
            av = av_pool.tile([128, 512], f32, name="av")
            cent3 = pbsb.tile([128, NB, D], bf16, name="cent3")
            mv = pbsb.tile([128, NB, 2], f32, name="mv")
            stt = pbsb.tile([128, NB, 6], f32, name="stt")
            osb = pbsb.tile([128, NB, DOUT], f32, name="osb")

            # ---- input DMAs: bf16 constants first on SP (weights gate both
            # matmul chains), x2 group 0 on the Pool queue, x1 halves on the
            # DVE queue — three queues issue in parallel ----
            cb = consts.tile([128, CB_W], bf16)
            nc.sync.dma_start(out=cb, in_=cb_d[:])

            def dma_x2(g, eng=None):
                t = x2load.tile([128, 4, 128], f32, tag="x2")
                (eng or nc.sync).dma_start(
                    out=t,
                    in_=x2[g * 512:(g + 1) * 512, :].rearrange(
                        "(p t) d -> p t d", p=128
                    ),
                )
                return t

            x2_pre = {0: dma_x2(0, nc.gpsimd)}
            x1h = []
            for half in range(2):
                t = x2load.tile([128, 4, 128], f32, tag="x2")
                nc.sync.dma_start(
                    out=t,
                    in_=x1[half * 512:(half + 1) * 512, :].rearrange(
                        "(t p) d -> p t d", p=128
                    ),
                )
                x1h.append(t)
            cf = consts.tile([128, CF_W], f32)
            nc.sync.dma_start(out=cf, in_=cf_d[:])
            identb = cb[:, CB_ID:CB_ID + 128]
            wq_b = cb[:, CB_WQ:CB_WQ + D]
            wk_b = cb[:, CB_WK:CB_WK + D]
            wv_b = cb[:, CB_WV:CB_WV + D]
            wog_b = cb[0:D, CB_WOG:CB_WOG + DOUT]
            bvb = cf[:, CF_BVB:CF_BVB + 128]
            biasb = cf[:, CF_BIAS:CF_BIAS + DOUT]
            bq_sb = cf[0:D, CF_BQ:CF_BQ + 1]
            bk_sb = cf[0:D, CF_BK:CF_BK + 1]
            epsc = cf[:, CF_EPS:CF_EPS + 1]

            # Pull the exp table load (~1.3us) into the initial DMA window.
            warm = consts.tile([1, 8], bf16)
            nc.scalar.activation(warm, cb[0:1, 0:8], AF.Exp)

            # ---- q prep: x1 -> bf16 -> x1T -> qT (+bq), per half;
            # bf16 rounding runs on the otherwise-idle ACT engine ----
            for half in range(2):
                x1_b = xbp.tile([128, 4, 128], bf16, tag="xb")
                nc.scalar.copy(x1_b, x1h[half])
                tps = tp_ps.tile([128, 768], bf16, tag="tp")
                for j in range(4):
                    nc.tensor.transpose(
                        tps[:, 128 * j:128 * (j + 1)], x1_b[:, j, :], identb
                    )
                nc.vector.tensor_copy(
                    x1t[:, 512 * half:512 * (half + 1)], tps[:, 0:512]
                )
                qps = sc_ps.tile([128, 1024], f32, tag="sc")
                nc.tensor.matmul(
                    qps[0:D, 0:512],
                    lhsT=wq_b,
                    rhs=x1t[:, 512 * half:512 * (half + 1)],
                    start=True,
                    stop=True,
                )
                nc.vector.tensor_scalar_add(
                    qt[:, 512 * half:512 * (half + 1)], qps[0:D, 0:512], bq_sb
                )

            # ---- k/v prep for one group of 4 chunks (512 kv rows) ----
            def prep_group(g):
                x2_sb = x2_pre.pop(g) if g in x2_pre else dma_x2(g)
                x2_b = xbp.tile([128, 4, 128], bf16, tag="xb")
                # ACT is idle before the first exp — round group 0 there
                (nc.scalar.copy if g == 0 else nc.vector.tensor_copy)(
                    x2_b, x2_sb
                )
                tps = tp_ps.tile([128, 768], bf16, tag="tp")
                for j in range(4):
                    nc.tensor.transpose(
                        tps[:, 128 * j:128 * (j + 1)], x2_b[:, j, :], identb
                    )
                x2t = x2tp.tile([128, 512], bf16, tag="x2t")
                nc.vector.tensor_copy(x2t, tps[:, 0:512])
                kps = kv_ps.tile([D, 512], f32, tag="kv")
                nc.tensor.matmul(kps, lhsT=wk_b, rhs=x2t, start=True, stop=True)
                nc.vector.tensor_scalar_add(
                    kstack[:, 512 * g:512 * (g + 1)], kps, bk_sb
                )
                # v chunks, natural [n, 32] -> f32 view of tp cols 512:768
                vview = tps.bitcast(f32)  # [128, 384] f32
                for j in range(4):
                    nc.tensor.matmul(
                        vview[:, 256 + 32 * j:256 + 32 * (j + 1)],
                        lhsT=x2t[:, 128 * j:128 * (j + 1)],
                        rhs=wv_b,
                        start=True,
                        stop=True,
                    )
                nc.vector.tensor_add(
                    vstack[:, 128 * g:128 * (g + 1)], vview[:, 256:384], bvb
                )

            def issue_scores(p, g, h):
                m0 = p * 512
                sps = sc_ps.tile([128, 1024], f32, tag="sc")
                for ci in range(2):
                    c = 4 * g + 2 * h + ci
                    nc.tensor.matmul(
                        sps[:, 512 * ci:512 * (ci + 1)],
                        lhsT=kstack[:, 128 * c:128 * (c + 1)],
                        rhs=qt[:, m0:m0 + 512],
                        start=True,
                        stop=True,
                    )
                pt = ptp.tile([128, 1024], bf16, tag="pt")
                nc.scalar.activation(pt, sps, AF.Exp)
                return pt

            first_av = [True]

            def issue_av(p, g, h, pt):
                for ci in range(2):
                    c = 4 * g + 2 * h + ci
                    for u in range(4):
                        b = 4 * p + u
                        # Exactly one start (global first) and one stop
                        # (global last): a start marks the whole PSUM
                        # zero-region pending-zero for every touched
                        # partition, so a second start would wipe other
                        # blocks' partial sums.  Each block's first touch
                        # still lands as overwrite via pending-zero bytes.
                        nc.tensor.matmul(
                            av[:, 32 * b:32 * (b + 1)],
                            lhsT=pt[:, 512 * ci + 128 * u:512 * ci + 128 * (u + 1)],
                            rhs=vstack[:, D * c:D * (c + 1)],
                            start=first_av[0],
                            stop=(c == NCH - 1 and p == 1 and u == 3),
                            skip_group_check=True,
                        )
                        first_av[0] = False

            def stats_block(b):
                nc.vector.bn_stats(out=stt[:, b, :], in_=av[:, D * b:D * (b + 1)])
                nc.vector.bn_aggr(out=mv[:, b, :], in_=stt[:, b, :])
                nc.vector.tensor_scalar(
                    cent3[:, b, :], av[:, D * b:D * (b + 1)], mv[:, b, 0:1],
                    None, op0=OP.subtract,
                )

            # ---- attention, m-pass outer; AV pipelined one tile behind so
            # the in-order PE never blocks the ACT exp stream ----
            pending = None
            for p in range(2):
                for g in range(NG):
                    if p == 0:
                        prep_group(g)
                    for h in range(2):
                        pt = issue_scores(p, g, h)
                        if pending is not None:
                            issue_av(*pending)
                        pending = (p, g, h, pt)
                if p == 0:
                    # first-half LN stats overlap the second attention pass
                    # (issue the straggler AV first so av cols 0:128 are done)
                    issue_av(*pending)
                    pending = None
                    for b in range(4):
                        stats_block(b)
            issue_av(*pending)

            if debug:
                nc.sync.dma_start(out=dbg_qt[:], in_=qt)
                nc.sync.dma_start(out=dbg_k[:], in_=kstack)
                nc.sync.dma_start(out=dbg_v[:], in_=vstack)
                avst = x2tp.tile([128, NB * D], f32, tag="avdbg")
                nc.vector.tensor_copy(avst, av[:, 0:NB * D])
                nc.sync.dma_start(out=dbg_av[:], in_=avst)

          # ---- phase B tail: stats for the second half, rstd, project ----
          with (
            tc.tile_pool(name="pb_ps", bufs=2, space="PSUM") as pb_ps,
            tc.tile_pool(name="naug", bufs=2) as naugp,
            tc.tile_pool(name="lnagg", bufs=8) as lnagg,
          ):
            magic_f = lnagg.tile([128, NB], f32, name="magicf")
            nc.vector.memset(magic_f, float(np.int32(0x5F3759DF).view(np.float32)))
            magic_i = magic_f.bitcast(mybir.dt.int32)
            for b in range(4, NB):
                stats_block(b)
            # rstd = 1/sqrt(var + eps) via fast-inverse-sqrt + two Newton
            # steps, all in DVE core-ISA ops (mult/subtract/shift) — avoids
            # the ~1.3us ACT Ln-table load on the tail critical path.
            # Signs: each Newton step computes -x_{n+1} from +x_n (the 1.5
            # constant enters negated), so after an even number of steps the
            # result is positive again.
            i32 = mybir.dt.int32
            ve = lnagg.tile([128, NB], f32)
            nc.vector.tensor_scalar_add(ve, mv[:, :, 1], LN_EPS)
            half_i = lnagg.tile([128, NB], i32)
            nc.vector.tensor_scalar(
                half_i, ve.bitcast(i32), 1, None, op0=OP.arith_shift_right
            )
            x0 = lnagg.tile([128, NB], f32)
            nc.vector.tensor_tensor(
                x0.bitcast(i32), magic_i, half_i, op=OP.subtract
            )
            cur = x0
            for it in range(2):
                sq = lnagg.tile([128, NB], f32, name=f"nsq{it}")
                nc.vector.tensor_tensor(sq, cur, cur, op=OP.mult)
                ysq = lnagg.tile([128, NB], f32, name=f"nys{it}")
                nc.vector.tensor_tensor(ysq, ve, sq, op=OP.mult)
                coef = lnagg.tile([128, NB], f32, name=f"ncf{it}")
                nc.vector.tensor_scalar(
                    coef, ysq, 0.5, 1.5, op0=OP.mult, op1=OP.subtract
                )
                nxt = lnagg.tile([128, NB], f32, name=f"nxt{it}")
                nc.vector.tensor_tensor(nxt, cur, coef, op=OP.mult)
                cur = nxt
            rstd = cur
            for b in range(NB):
                nps = pb_ps.tile([D, 128], bf16, tag="pbt")
                nc.tensor.transpose(nps, cent3[:, b, :], identb)
                na = naugp.tile([D, 128], bf16, tag="na")
                # ACT is idle in the tail; GPSIMD cannot read PSUM
                nc.scalar.copy(na, nps)
                ops = pb_ps.tile([128, DOUT], f32, tag="pb")
                nc.tensor.matmul(ops, lhsT=na, rhs=wog_b, start=True, stop=True)
                # out = rstd * (cent @ gWo) + (beta @ Wo + bo)
                nc.vector.scalar_tensor_tensor(
                    out=osb[:, b, :], in0=ops, scalar=rstd[:, b:b + 1],
                    in1=biasb, op0=OP.mult, op1=OP.add,
                )
                if b % 2 == 1:
                    nc.sync.dma_start(
                        out=out_d[(b - 1) * 128:(b + 1) * 128, :].rearrange(
                            "(b p) d -> p b d", p=128
                        ),
                        in_=osb[:, b - 1:b + 1, :],
                    )

    nc.compile()
    _CACHE[key] = nc
    return nc


def host_inputs(input1, input2, Wq, bq, Wk, bk, Wv, bv, gamma, beta, Wo, bo):
    """Per-core input maps (host-side weight folding + constant packing)."""
    f32 = np.float32
    input1 = np.ascontiguousarray(np.asarray(input1, f32))
    input2 = np.ascontiguousarray(np.asarray(input2, f32))
    Wo = np.asarray(Wo, f32)
    try:
        import ml_dtypes
        bf = ml_dtypes.bfloat16
    except ImportError:  # pragma: no cover
        import jax.numpy as jnp
        bf = jnp.bfloat16
    cb = np.zeros((128, CB_W), f32)
    cb[:, CB_ID:CB_ID + 128] = np.eye(128, dtype=f32)
    cb[:, CB_WQ:CB_WQ + D] = np.asarray(Wq, f32)
    cb[:, CB_WK:CB_WK + D] = np.asarray(Wk, f32)
    cb[:, CB_WV:CB_WV + D] = np.asarray(Wv, f32)
    cb[0:D, CB_WOG:CB_WOG + DOUT] = np.asarray(gamma, f32)[:, None] * Wo
    cf = np.zeros((128, CF_W), f32)
    cf[:, CF_BVB:CF_BVB + 128] = np.tile(np.asarray(bv, f32), 4)[None, :]
    cf[:, CF_BIAS:CF_BIAS + DOUT] = (
        np.asarray(beta, f32) @ Wo + np.asarray(bo, f32)
    )[None, :]
    cf[0:D, CF_BQ] = np.asarray(bq, f32)
    cf[0:D, CF_BK] = np.asarray(bk, f32)
    cf[:, CF_EPS] = LN_EPS
    common = {"x2": input2, "cpackb": cb.astype(bf), "cpackf": cf}
    return [
        dict(common, x1=input1[c * MSH:(c + 1) * MSH]) for c in range(NCORES)
    ]


def kernel(input1, input2, Wq, bq, Wk, bk, Wv, bv, gamma, beta, Wo, bo):
    _import_concourse()
    from concourse.bass_utils import run_bass_kernel_spmd

    nc = build()
    in_maps = host_inputs(
        input1, input2, Wq, bq, Wk, bk, Wv, bv, gamma, beta, Wo, bo
    )
    res = run_bass_kernel_spmd(nc, in_maps, list(range(NCORES)))
    return np.concatenate(
        [np.asarray(res.results[c]["out"]) for c in range(NCORES)], axis=0
    ).astype(np.float32)


# revision 45
# speedup vs baseline: 1.0140x; 1.0140x over previous
"""Trainium2 Bass kernel for fused attention + LayerNorm + projection.

Computation (per reference):
    q = input1 @ Wq + bq                       [8192, 32]
    k = input2 @ Wk + bk                       [8192, 32]
    v = input2 @ Wv + bv                       [8192, 32]
    P = softmax(q @ k.T, axis=-1)              [8192, 8192]
    fused = P @ v                              [8192, 32]
    out = LayerNorm(fused) * gamma + beta @ Wo + bo   [8192, 128]

Sharding: data-parallel over rows of input1 (1024 rows per core, 8 cores);
input2 and weights replicated.

Algebraic simplifications (host + device):
  - softmax normalization (and max-subtraction) is skipped: LayerNorm is
    invariant to a positive per-row scale, so exp(s) @ v is normalized for
    free by LN (eps term differs by ~1e-3 relative — validated vs reference).
  - gamma is folded into Wo (diag(gamma) @ Wo) on the host; the constant
    output row beta @ Wo + bo is added at the end.  The per-row 1/std of
    LayerNorm commutes with the projection, so it is applied to the
    *projected* row (out = rstd * (cent @ gWo) + bias) — this takes the
    ACT-table-loading rstd computation off the critical path of phase B.

Engine plan (all matmuls stream at 1 cycle/column):
  - The whole q/k/v/scores/AV matmul pipeline runs in bf16: DVE rounds
    f32 DMA data to bf16, PE transposes bf16 (1 cyc/row vs 2-4 for f32),
    k^T is one group-wide [32,512] matmul per 512 kv rows.
  - AV runs in natural orientation: out[m-block 128, 32] accumulates in
    PSUM over all 64 kv chunks with lhsT = exp-scores [128 n, 128 m] and
    rhs = v [128 n, 32]: 32-cycle matmuls at full 128-partition
    contraction (16k PE cycles vs 65k transposed), and `fused` lands
    natural so LayerNorm needs no transposes.  The PSUM accumulator uses
    exactly ONE start flag (hardware marks the whole 2KB zero-region
    pending-zero on start — a second start would wipe other blocks).
  - exp runs on ACT straight out of PSUM into bf16 SBUF tiles; at
    8.4M elements/core ACT is the pacing engine (~71 us busy), PE
    (~45 us) and DVE (~40 us) hide under it.
  - m-passes are outer (p=0 rows 0:512 with k/v prep, then p=1): phase-B
    stats/centering for the first half overlap the second attention pass.
  - All small constants ship as ONE packed [128, 611] DMA + one bulk
    bf16 conversion; input1 loads as two half DMAs so the q-prep chain
    starts sooner; output stages into one [128, 8, 128] tile and leaves
    as ONE DMA.
"""

import os
import sys

import numpy as np

N1 = 8192
N2 = 8192
DIN = 128
D = 32
DOUT = 128
NCORES = 8
MSH = N1 // NCORES          # rows per core
NCH = N2 // 128             # 64 in2 chunks
NG = NCH // 4               # 16 groups of 4 chunks
NB = MSH // 128             # 8 output m-blocks
LN_EPS = 1e-5

# packed-constant column layouts: bf16 block (matmul operands, shipped
# pre-converted) and f32 block (bias/eps scalars)
CB_ID = 0
CB_WQ = 128
CB_WK = 160
CB_WV = 192
CB_WOG = 224
CB_W = 352
CF_BVB = 0
CF_BIAS = 128
CF_BQ = 256
CF_BK = 257
CF_EPS = 258
CF_W = 259

_CACHE = {}


def _import_concourse():
    try:
        import concourse.bass  # noqa: F401
    except ImportError:
        for p in ("/opt/trn_rl_repo", os.path.expanduser("~/.axon_site/_ro/trn_rl_repo")):
            if os.path.isdir(p) and p not in sys.path:
                sys.path.insert(0, p)


def build(reps=1):
    """Build (and cache) the compiled single-core SPMD Bass program.

    reps > 1 repeats the whole computation serially (for wall-clock slope
    timing); the output is rewritten identically each rep.
    """
    key = ("nc", reps)
    if key in _CACHE:
        return _CACHE[key]
    _import_concourse()
    import concourse.bacc as bacc
    import concourse.tile as tile
    from concourse import mybir

    f32 = mybir.dt.float32
    bf16 = mybir.dt.bfloat16
    AF = mybir.ActivationFunctionType
    OP = mybir.AluOpType

    nc = bacc.Bacc(None, target_bir_lowering=False, debug=False)

    x1 = nc.dram_tensor("x1", [MSH, DIN], f32, kind="ExternalInput")
    x2 = nc.dram_tensor("x2", [N2, DIN], f32, kind="ExternalInput")
    cb_d = nc.dram_tensor("cpackb", [128, CB_W], bf16, kind="ExternalInput")
    cf_d = nc.dram_tensor("cpackf", [128, CF_W], f32, kind="ExternalInput")
    out_d = nc.dram_tensor("out", [MSH, DOUT], f32, kind="ExternalOutput")
    debug = bool(os.environ.get("KDEBUG"))
    if debug:
        dbg_qt = nc.dram_tensor("dbg_qt", [D, MSH], bf16, kind="ExternalOutput")
        dbg_k = nc.dram_tensor("dbg_k", [D, NCH * 128], bf16, kind="ExternalOutput")
        dbg_v = nc.dram_tensor("dbg_v", [128, NCH * D], bf16, kind="ExternalOutput")
        dbg_av = nc.dram_tensor("dbg_av", [128, NB * D], f32, kind="ExternalOutput")

    from contextlib import ExitStack

    with tile.TileContext(nc) as tc, ExitStack() as outer:
        consts = outer.enter_context(tc.tile_pool(name="consts", bufs=1))
        persist = outer.enter_context(tc.tile_pool(name="persist", bufs=1))
        av_pool = outer.enter_context(
            tc.tile_pool(name="av_ps", bufs=1, space="PSUM")
        )
        # phase-B SBUF state lives at rep scope so the first-half stats can
        # overlap the second attention pass inside the phase-A pool scope
        pbsb = outer.enter_context(tc.tile_pool(name="pbsb", bufs=1))

        kstack = persist.tile([D, NCH * 128], bf16)   # kT chunk c at cols 128c
        vstack = persist.tile([128, NCH * D], bf16)   # v chunk c at cols 32c
        qt = persist.tile([D, MSH], bf16)             # qT, single band
        x1t = persist.tile([128, MSH], bf16)          # input1 shard transposed

        for _rep in range(reps):
          with (
            tc.tile_pool(name="x2load", bufs=3) as x2load,
            tc.tile_pool(name="xb", bufs=2) as xbp,
            tc.tile_pool(name="x2t_sb", bufs=2) as x2tp,
            tc.tile_pool(name="tp_ps", bufs=2, space="PSUM") as tp_ps,
            tc.tile_pool(name="kv_ps", bufs=1, space="PSUM") as kv_ps,
            tc.tile_pool(name="sc_ps", bufs=2, space="PSUM") as sc_ps,
            tc.tile_pool(name="pt", bufs=4) as ptp,
          ):
            av = av_pool.tile([128, 512], f32, name="av")
            cent3 = pbsb.tile([128, NB, D], bf16, name="cent3")
            mv = pbsb.tile([128, NB, 2], f32, name="mv")
            stt = pbsb.tile([128, NB, 6], f32, name="stt")
            osb = pbsb.tile([128, NB, DOUT], f32, name="osb")

            # ---- input DMAs: bf16 constants first on SP (weights gate both
            # matmul chains), x2 group 0 on the Pool queue, x1 halves on the
            # DVE queue — three queues issue in parallel ----
            cb = consts.tile([128, CB_W], bf16)
            nc.sync.dma_start(out=cb, in_=cb_d[:])

            def dma_x2(g, eng=None):
                t = x2load.tile([128, 4, 128], f32, tag="x2")
                (eng or nc.sync).dma_start(
                    out=t,
                    in_=x2[g * 512:(g + 1) * 512, :].rearrange(
                        "(p t) d -> p t d", p=128
                    ),
                )
                return t

            x2_pre = {0: dma_x2(0, nc.gpsimd)}
            x1h = []
            for half in range(2):
                t = x2load.tile([128, 4, 128], f32, tag="x2")
                nc.sync.dma_start(
                    out=t,
                    in_=x1[half * 512:(half + 1) * 512, :].rearrange(
                        "(t p) d -> p t d", p=128
                    ),
                )
                x1h.append(t)
            cf = consts.tile([128, CF_W], f32)
            nc.sync.dma_start(out=cf, in_=cf_d[:])
            identb = cb[:, CB_ID:CB_ID + 128]
            wq_b = cb[:, CB_WQ:CB_WQ + D]
            wk_b = cb[:, CB_WK:CB_WK + D]
            wv_b = cb[:, CB_WV:CB_WV + D]
            wog_b = cb[0:D, CB_WOG:CB_WOG + DOUT]
            bvb = cf[:, CF_BVB:CF_BVB + 128]
            biasb = cf[:, CF_BIAS:CF_BIAS + DOUT]
            bq_sb = cf[0:D, CF_BQ:CF_BQ + 1]
            bk_sb = cf[0:D, CF_BK:CF_BK + 1]
            epsc = cf[:, CF_EPS:CF_EPS + 1]

            # Pull the exp table load (~1.3us) into the initial DMA window.
            warm = consts.tile([1, 8], bf16)
            nc.scalar.activation(warm, cb[0:1, 0:8], AF.Exp)

            # ---- q prep: x1 -> bf16 -> x1T -> qT (+bq), per half;
            # bf16 rounding runs on the otherwise-idle ACT engine ----
            for half in range(2):
                x1_b = xbp.tile([128, 4, 128], bf16, tag="xb")
                nc.scalar.copy(x1_b, x1h[half])
                tps = tp_ps.tile([128, 768], bf16, tag="tp")
                for j in range(4):
                    nc.tensor.transpose(
                        tps[:, 128 * j:128 * (j + 1)], x1_b[:, j, :], identb
                    )
                nc.vector.tensor_copy(
                    x1t[:, 512 * half:512 * (half + 1)], tps[:, 0:512]
                )
                qps = sc_ps.tile([128, 1024], f32, tag="sc")
                nc.tensor.matmul(
                    qps[0:D, 0:512],
                    lhsT=wq_b,
                    rhs=x1t[:, 512 * half:512 * (half + 1)],
                    start=True,
                    stop=True,
                )
                nc.vector.tensor_scalar_add(
                    qt[:, 512 * half:512 * (half + 1)], qps[0:D, 0:512], bq_sb
                )

            # ---- k/v prep for one group of 4 chunks (512 kv rows) ----
            def prep_group(g):
                x2_sb = x2_pre.pop(g) if g in x2_pre else dma_x2(g)
                x2_b = xbp.tile([128, 4, 128], bf16, tag="xb")
                # ACT is idle before the first exp — round group 0 there
                (nc.scalar.copy if g == 0 else nc.vector.tensor_copy)(
                    x2_b, x2_sb
                )
                tps = tp_ps.tile([128, 768], bf16, tag="tp")
                for j in range(4):
                    nc.tensor.transpose(
                        tps[:, 128 * j:128 * (j + 1)], x2_b[:, j, :], identb
                    )
                x2t = x2tp.tile([128, 512], bf16, tag="x2t")
                nc.vector.tensor_copy(x2t, tps[:, 0:512])
                kps = kv_ps.tile([D, 512], f32, tag="kv")
                nc.tensor.matmul(kps, lhsT=wk_b, rhs=x2t, start=True, stop=True)
                nc.vector.tensor_scalar_add(
                    kstack[:, 512 * g:512 * (g + 1)], kps, bk_sb
                )
                # v chunks, natural [n, 32] -> f32 view of tp cols 512:768
                vview = tps.bitcast(f32)  # [128, 384] f32
                for j in range(4):
                    nc.tensor.matmul(
                        vview[:, 256 + 32 * j:256 + 32 * (j + 1)],
                        lhsT=x2t[:, 128 * j:128 * (j + 1)],
                        rhs=wv_b,
                        start=True,
                        stop=True,
                    )
                nc.vector.tensor_add(
                    vstack[:, 128 * g:128 * (g + 1)], vview[:, 256:384], bvb
                )

            def issue_scores(p, g, h):
                m0 = p * 512
                sps = sc_ps.tile([128, 1024], f32, tag="sc")
                for ci in range(2):
                    c = 4 * g + 2 * h + ci
                    nc.tensor.matmul(
                        sps[:, 512 * ci:512 * (ci + 1)],
                        lhsT=kstack[:, 128 * c:128 * (c + 1)],
                        rhs=qt[:, m0:m0 + 512],
                        start=True,
                        stop=True,
                    )
                pt = ptp.tile([128, 1024], bf16, tag="pt")
                nc.scalar.activation(pt, sps, AF.Exp)
                return pt

            first_av = [True]

            def issue_av(p, g, h, pt):
                for ci in range(2):
                    c = 4 * g + 2 * h + ci
                    for u in range(4):
                        b = 4 * p + u
                        # Exactly one start (global first) and one stop
                        # (global last): a start marks the whole PSUM
                        # zero-region pending-zero for every touched
                        # partition, so a second start would wipe other
                        # blocks' partial sums.  Each block's first touch
                        # still lands as overwrite via pending-zero bytes.
                        nc.tensor.matmul(
                            av[:, 32 * b:32 * (b + 1)],
                            lhsT=pt[:, 512 * ci + 128 * u:512 * ci + 128 * (u + 1)],
                            rhs=vstack[:, D * c:D * (c + 1)],
                            start=first_av[0],
                            stop=(c == NCH - 1 and p == 1 and u == 3),
                            skip_group_check=True,
                        )
                        first_av[0] = False

            def stats_block(b):
                nc.vector.bn_stats(out=stt[:, b, :], in_=av[:, D * b:D * (b + 1)])
                nc.vector.bn_aggr(out=mv[:, b, :], in_=stt[:, b, :])
                nc.vector.tensor_scalar(
                    cent3[:, b, :], av[:, D * b:D * (b + 1)], mv[:, b, 0:1],
                    None, op0=OP.subtract,
                )

            # ---- attention, m-pass outer; AV pipelined one tile behind so
            # the in-order PE never blocks the ACT exp stream ----
            pending = None
            for p in range(2):
                for g in range(NG):
                    if p == 0:
                        prep_group(g)
                    for h in range(2):
                        pt = issue_scores(p, g, h)
                        if pending is not None:
                            issue_av(*pending)
                        pending = (p, g, h, pt)
                if p == 0:
                    # first-half LN stats overlap the second attention pass
                    # (issue the straggler AV first so av cols 0:128 are done)
                    issue_av(*pending)
                    pending = None
                    for b in range(4):
                        stats_block(b)
            issue_av(*pending)

            if debug:
                nc.sync.dma_start(out=dbg_qt[:], in_=qt)
                nc.sync.dma_start(out=dbg_k[:], in_=kstack)
                nc.sync.dma_start(out=dbg_v[:], in_=vstack)
                avst = x2tp.tile([128, NB * D], f32, tag="avdbg")
                nc.vector.tensor_copy(avst, av[:, 0:NB * D])
                nc.sync.dma_start(out=dbg_av[:], in_=avst)

          # ---- phase B tail: stats for the second half, rstd, project ----
          with (
            tc.tile_pool(name="pb_ps", bufs=2, space="PSUM") as pb_ps,
            tc.tile_pool(name="naug", bufs=2) as naugp,
            tc.tile_pool(name="lnagg", bufs=8) as lnagg,
          ):
            magic_f = lnagg.tile([128, NB], f32, name="magicf")
            nc.vector.memset(magic_f, float(np.int32(0x5F3759DF).view(np.float32)))
            magic_i = magic_f.bitcast(mybir.dt.int32)
            for b in range(4, NB):
                stats_block(b)
            # rstd = 1/sqrt(var + eps) via fast-inverse-sqrt + two Newton
            # steps, all in DVE core-ISA ops (mult/subtract/shift) — avoids
            # the ~1.3us ACT Ln-table load on the tail critical path.
            # Signs: each Newton step computes -x_{n+1} from +x_n (the 1.5
            # constant enters negated), so after an even number of steps the
            # result is positive again.
            i32 = mybir.dt.int32
            ve = lnagg.tile([128, NB], f32)
            nc.vector.tensor_scalar_add(ve, mv[:, :, 1], LN_EPS)
            half_i = lnagg.tile([128, NB], i32)
            nc.vector.tensor_scalar(
                half_i, ve.bitcast(i32), 1, None, op0=OP.arith_shift_right
            )
            x0 = lnagg.tile([128, NB], f32)
            nc.vector.tensor_tensor(
                x0.bitcast(i32), magic_i, half_i, op=OP.subtract
            )
            cur = x0
            for it in range(2):
                sq = lnagg.tile([128, NB], f32, name=f"nsq{it}")
                nc.vector.tensor_tensor(sq, cur, cur, op=OP.mult)
                ysq = lnagg.tile([128, NB], f32, name=f"nys{it}")
                nc.vector.tensor_tensor(ysq, ve, sq, op=OP.mult)
                coef = lnagg.tile([128, NB], f32, name=f"ncf{it}")
                nc.vector.tensor_scalar(
                    coef, ysq, 0.5, 1.5, op0=OP.mult, op1=OP.subtract
                )
                nxt = lnagg.tile([128, NB], f32, name=f"nxt{it}")
                nc.vector.tensor_tensor(nxt, cur, coef, op=OP.mult)
                cur = nxt
            rstd = cur
            for b in range(NB):
                nps = pb_ps.tile([D, 128], bf16, tag="pbt")
                nc.tensor.transpose(nps, cent3[:, b, :], identb)
                na = naugp.tile([D, 128], bf16, tag="na")
                # ACT is idle in the tail; GPSIMD cannot read PSUM
                nc.scalar.copy(na, nps)
                ops = pb_ps.tile([128, DOUT], f32, tag="pb")
                nc.tensor.matmul(ops, lhsT=na, rhs=wog_b, start=True, stop=True)
                # out = rstd * (cent @ gWo) + (beta @ Wo + bo)
                nc.vector.scalar_tensor_tensor(
                    out=osb[:, b, :], in0=ops, scalar=rstd[:, b:b + 1],
                    in1=biasb, op0=OP.mult, op1=OP.add,
                )
                if b % 2 == 1:
                    nc.sync.dma_start(
                        out=out_d[(b - 1) * 128:(b + 1) * 128, :].rearrange(
                            "(b p) d -> p b d", p=128
                        ),
                        in_=osb[:, b - 1:b + 1, :],
                    )

    nc.compile()
    _CACHE[key] = nc
    return nc


def host_inputs(input1, input2, Wq, bq, Wk, bk, Wv, bv, gamma, beta, Wo, bo):
    """Per-core input maps (host-side weight folding + constant packing)."""
    f32 = np.float32
    input1 = np.ascontiguousarray(np.asarray(input1, f32))
    input2 = np.ascontiguousarray(np.asarray(input2, f32))
    Wo = np.asarray(Wo, f32)
    try:
        import ml_dtypes
        bf = ml_dtypes.bfloat16
    except ImportError:  # pragma: no cover
        import jax.numpy as jnp
        bf = jnp.bfloat16
    cb = np.zeros((128, CB_W), f32)
    cb[:, CB_ID:CB_ID + 128] = np.eye(128, dtype=f32)
    cb[:, CB_WQ:CB_WQ + D] = np.asarray(Wq, f32)
    cb[:, CB_WK:CB_WK + D] = np.asarray(Wk, f32)
    cb[:, CB_WV:CB_WV + D] = np.asarray(Wv, f32)
    cb[0:D, CB_WOG:CB_WOG + DOUT] = np.asarray(gamma, f32)[:, None] * Wo
    cf = np.zeros((128, CF_W), f32)
    cf[:, CF_BVB:CF_BVB + 128] = np.tile(np.asarray(bv, f32), 4)[None, :]
    cf[:, CF_BIAS:CF_BIAS + DOUT] = (
        np.asarray(beta, f32) @ Wo + np.asarray(bo, f32)
    )[None, :]
    cf[0:D, CF_BQ] = np.asarray(bq, f32)
    cf[0:D, CF_BK] = np.asarray(bk, f32)
    cf[:, CF_EPS] = LN_EPS
    common = {"x2": input2, "cpackb": cb.astype(bf), "cpackf": cf}
    return [
        dict(common, x1=input1[c * MSH:(c + 1) * MSH]) for c in range(NCORES)
    ]


def kernel(input1, input2, Wq, bq, Wk, bk, Wv, bv, gamma, beta, Wo, bo):
    _import_concourse()
    from concourse.bass_utils import run_bass_kernel_spmd

    nc = build()
    in_maps = host_inputs(
        input1, input2, Wq, bq, Wk, bk, Wv, bv, gamma, beta, Wo, bo
    )
    res = run_bass_kernel_spmd(nc, in_maps, list(range(NCORES)))
    return np.concatenate(
        [np.asarray(res.results[c]["out"]) for c in range(NCORES)], axis=0
    ).astype(np.float32)


# revision 48
# speedup vs baseline: 1.0449x; 1.0304x over previous
"""Trainium2 Bass kernel for fused attention + LayerNorm + projection.

Computation (per reference):
    q = input1 @ Wq + bq                       [8192, 32]
    k = input2 @ Wk + bk                       [8192, 32]
    v = input2 @ Wv + bv                       [8192, 32]
    P = softmax(q @ k.T, axis=-1)              [8192, 8192]
    fused = P @ v                              [8192, 32]
    out = LayerNorm(fused) * gamma + beta @ Wo + bo   [8192, 128]

Sharding: data-parallel over rows of input1 (1024 rows per core, 8 cores);
input2 and weights replicated.

Algebraic simplifications (host + device):
  - softmax normalization (and max-subtraction) is skipped: LayerNorm is
    invariant to a positive per-row scale, so exp(s) @ v is normalized for
    free by LN (eps term differs by ~1e-3 relative — validated vs reference).
  - gamma is folded into Wo (diag(gamma) @ Wo) on the host; the constant
    output row beta @ Wo + bo is added at the end.  The per-row 1/std of
    LayerNorm commutes with the projection, so it is applied to the
    *projected* row (out = rstd * (cent @ gWo) + bias) — this takes the
    ACT-table-loading rstd computation off the critical path of phase B.

Engine plan (all matmuls stream at 1 cycle/column):
  - The whole q/k/v/scores/AV matmul pipeline runs in bf16: DVE rounds
    f32 DMA data to bf16, PE transposes bf16 (1 cyc/row vs 2-4 for f32),
    k^T is one group-wide [32,512] matmul per 512 kv rows.
  - AV runs in natural orientation: out[m-block 128, 32] accumulates in
    PSUM over all 64 kv chunks with lhsT = exp-scores [128 n, 128 m] and
    rhs = v [128 n, 32]: 32-cycle matmuls at full 128-partition
    contraction (16k PE cycles vs 65k transposed), and `fused` lands
    natural so LayerNorm needs no transposes.  The PSUM accumulator uses
    exactly ONE start flag (hardware marks the whole 2KB zero-region
    pending-zero on start — a second start would wipe other blocks).
  - exp runs on ACT straight out of PSUM into bf16 SBUF tiles; at
    8.4M elements/core ACT is the pacing engine (~71 us busy), PE
    (~45 us) and DVE (~40 us) hide under it.
  - m-passes are outer (p=0 rows 0:512 with k/v prep, then p=1): phase-B
    stats/centering for the first half overlap the second attention pass.
  - All small constants ship as ONE packed [128, 611] DMA + one bulk
    bf16 conversion; input1 loads as two half DMAs so the q-prep chain
    starts sooner; output stages into one [128, 8, 128] tile and leaves
    as ONE DMA.
"""

import os
import sys

import numpy as np

N1 = 8192
N2 = 8192
DIN = 128
D = 32
DOUT = 128
NCORES = 8
MSH = N1 // NCORES          # rows per core
NCH = N2 // 128             # 64 in2 chunks
NG = NCH // 4               # 16 groups of 4 chunks
NB = MSH // 128             # 8 output m-blocks
LN_EPS = 1e-5

# packed-constant column layouts: bf16 block (matmul operands, shipped
# pre-converted) and f32 block (bias/eps scalars)
CB_ID = 0
CB_WQ = 128
CB_WK = 160
CB_WV = 192
CB_WOG = 224
CB_W = 352
CF_BVB = 0
CF_BIAS = 128
CF_BQ = 256
CF_BK = 257
CF_EPS = 258
CF_W = 259

_CACHE = {}


def _import_concourse():
    try:
        import concourse.bass  # noqa: F401
    except ImportError:
        for p in ("/opt/trn_rl_repo", os.path.expanduser("~/.axon_site/_ro/trn_rl_repo")):
            if os.path.isdir(p) and p not in sys.path:
                sys.path.insert(0, p)


def build(reps=1):
    """Build (and cache) the compiled single-core SPMD Bass program.

    reps > 1 repeats the whole computation serially (for wall-clock slope
    timing); the output is rewritten identically each rep.
    """
    key = ("nc", reps)
    if key in _CACHE:
        return _CACHE[key]
    _import_concourse()
    import concourse.bacc as bacc
    import concourse.tile as tile
    from concourse import mybir

    f32 = mybir.dt.float32
    bf16 = mybir.dt.bfloat16
    AF = mybir.ActivationFunctionType
    OP = mybir.AluOpType

    nc = bacc.Bacc(None, target_bir_lowering=False, debug=False)

    x1 = nc.dram_tensor("x1", [MSH, DIN], f32, kind="ExternalInput")
    x2 = nc.dram_tensor("x2", [N2, DIN], f32, kind="ExternalInput")
    cb_d = nc.dram_tensor("cpackb", [128, CB_W], bf16, kind="ExternalInput")
    cf_d = nc.dram_tensor("cpackf", [128, CF_W], f32, kind="ExternalInput")
    out_d = nc.dram_tensor("out", [MSH, DOUT], f32, kind="ExternalOutput")
    debug = bool(os.environ.get("KDEBUG"))
    if debug:
        dbg_qt = nc.dram_tensor("dbg_qt", [D, MSH], bf16, kind="ExternalOutput")
        dbg_k = nc.dram_tensor("dbg_k", [D, NCH * 128], bf16, kind="ExternalOutput")
        dbg_v = nc.dram_tensor("dbg_v", [128, NCH * D], bf16, kind="ExternalOutput")
        dbg_av = nc.dram_tensor("dbg_av", [128, NB * D], f32, kind="ExternalOutput")

    from contextlib import ExitStack

    with tile.TileContext(nc) as tc, ExitStack() as outer:
        consts = outer.enter_context(tc.tile_pool(name="consts", bufs=1))
        persist = outer.enter_context(tc.tile_pool(name="persist", bufs=1))
        av_pool = outer.enter_context(
            tc.tile_pool(name="av_ps", bufs=1, space="PSUM")
        )
        # phase-B SBUF state lives at rep scope so the first-half stats can
        # overlap the second attention pass inside the phase-A pool scope
        pbsb = outer.enter_context(tc.tile_pool(name="pbsb", bufs=1))

        kstack = persist.tile([D, NCH * 128], bf16)   # kT chunk c at cols 128c
        vstack = persist.tile([128, NCH * D], bf16)   # v chunk c at cols 32c
        qt = persist.tile([D, MSH], bf16)             # qT, single band
        x1t = persist.tile([128, MSH], bf16)          # input1 shard transposed

        for _rep in range(reps):
          with (
            tc.tile_pool(name="x2load", bufs=3) as x2load,
            tc.tile_pool(name="xb", bufs=2) as xbp,
            tc.tile_pool(name="x2t_sb", bufs=2) as x2tp,
            tc.tile_pool(name="tp_ps", bufs=2, space="PSUM") as tp_ps,
            tc.tile_pool(name="kv_ps", bufs=1, space="PSUM") as kv_ps,
            tc.tile_pool(name="sc_ps", bufs=2, space="PSUM") as sc_ps,
            tc.tile_pool(name="pt", bufs=4) as ptp,
          ):
            av = av_pool.tile([128, 512], f32, name="av")
            cent3 = pbsb.tile([128, NB, D], bf16, name="cent3")
            mv = pbsb.tile([128, NB, 2], f32, name="mv")
            stt = pbsb.tile([128, NB, 6], f32, name="stt")
            osb = pbsb.tile([128, NB, DOUT], f32, name="osb")

            # ---- input DMAs: bf16 constants first on SP (weights gate both
            # matmul chains), x2 group 0 on the Pool queue, x1 halves on the
            # DVE queue — three queues issue in parallel ----
            x1h = []
            t0h = x2load.tile([128, 4, 128], f32, tag="x2")
            nc.sync.dma_start(
                out=t0h, in_=x1[0:512, :].rearrange("(t p) d -> p t d", p=128)
            )
            x1h.append(t0h)
            cb = consts.tile([128, CB_W], bf16)
            nc.sync.dma_start(out=cb, in_=cb_d[:])

            def dma_x2(g, eng=None):
                t = x2load.tile([128, 4, 128], f32, tag="x2")
                (eng or nc.sync).dma_start(
                    out=t,
                    in_=x2[g * 512:(g + 1) * 512, :].rearrange(
                        "(p t) d -> p t d", p=128
                    ),
                )
                return t

            x2_pre = {0: dma_x2(0, nc.gpsimd)}
            t1h = x2load.tile([128, 4, 128], f32, tag="x2")
            nc.sync.dma_start(
                out=t1h, in_=x1[512:1024, :].rearrange("(t p) d -> p t d", p=128)
            )
            x1h.append(t1h)
            cf = consts.tile([128, CF_W], f32)
            nc.sync.dma_start(out=cf, in_=cf_d[:])
            identb = cb[:, CB_ID:CB_ID + 128]
            wq_b = cb[:, CB_WQ:CB_WQ + D]
            wk_b = cb[:, CB_WK:CB_WK + D]
            wv_b = cb[:, CB_WV:CB_WV + D]
            wog_b = cb[0:D, CB_WOG:CB_WOG + DOUT]
            bvb = cf[:, CF_BVB:CF_BVB + 128]
            biasb = cf[:, CF_BIAS:CF_BIAS + DOUT]
            bq_sb = cf[0:D, CF_BQ:CF_BQ + 1]
            bk_sb = cf[0:D, CF_BK:CF_BK + 1]
            epsc = cf[:, CF_EPS:CF_EPS + 1]

            # Pull the exp table load (~1.3us) into the initial DMA window.
            warm = consts.tile([1, 8], bf16)
            nc.scalar.activation(warm, cb[0:1, 0:8], AF.Exp)

            # ---- q prep: x1 -> bf16 -> x1T -> qT (+bq), per half;
            # bf16 rounding runs on the otherwise-idle ACT engine ----
            for half in range(2):
                x1_b = xbp.tile([128, 4, 128], bf16, tag="xb")
                nc.scalar.copy(x1_b, x1h[half])
                tps = tp_ps.tile([128, 768], bf16, tag="tp")
                for j in range(4):
                    nc.tensor.transpose(
                        tps[:, 128 * j:128 * (j + 1)], x1_b[:, j, :], identb
                    )
                nc.vector.tensor_copy(
                    x1t[:, 512 * half:512 * (half + 1)], tps[:, 0:512]
                )
                qps = sc_ps.tile([128, 1024], f32, tag="sc")
                nc.tensor.matmul(
                    qps[0:D, 0:512],
                    lhsT=wq_b,
                    rhs=x1t[:, 512 * half:512 * (half + 1)],
                    start=True,
                    stop=True,
                )
                nc.vector.tensor_scalar_add(
                    qt[:, 512 * half:512 * (half + 1)], qps[0:D, 0:512], bq_sb
                )

            # ---- k/v prep for one group of 4 chunks (512 kv rows) ----
            def prep_group(g):
                x2_sb = x2_pre.pop(g) if g in x2_pre else dma_x2(g)
                x2_b = xbp.tile([128, 4, 128], bf16, tag="xb")
                # ACT is idle before the first exp — round group 0 there
                (nc.scalar.copy if g == 0 else nc.vector.tensor_copy)(
                    x2_b, x2_sb
                )
                tps = tp_ps.tile([128, 768], bf16, tag="tp")
                for j in range(4):
                    nc.tensor.transpose(
                        tps[:, 128 * j:128 * (j + 1)], x2_b[:, j, :], identb
                    )
                x2t = x2tp.tile([128, 512], bf16, tag="x2t")
                nc.vector.tensor_copy(x2t, tps[:, 0:512])
                kps = kv_ps.tile([D, 512], f32, tag="kv")
                nc.tensor.matmul(kps, lhsT=wk_b, rhs=x2t, start=True, stop=True)
                nc.vector.tensor_scalar_add(
                    kstack[:, 512 * g:512 * (g + 1)], kps, bk_sb
                )
                # v chunks, natural [n, 32] -> f32 view of tp cols 512:768
                vview = tps.bitcast(f32)  # [128, 384] f32
                for j in range(4):
                    nc.tensor.matmul(
                        vview[:, 256 + 32 * j:256 + 32 * (j + 1)],
                        lhsT=x2t[:, 128 * j:128 * (j + 1)],
                        rhs=wv_b,
                        start=True,
                        stop=True,
                    )
                nc.vector.tensor_add(
                    vstack[:, 128 * g:128 * (g + 1)], vview[:, 256:384], bvb
                )

            def issue_scores(p, g, h):
                m0 = p * 512
                sps = sc_ps.tile([128, 1024], f32, tag="sc")
                for ci in range(2):
                    c = 4 * g + 2 * h + ci
                    nc.tensor.matmul(
                        sps[:, 512 * ci:512 * (ci + 1)],
                        lhsT=kstack[:, 128 * c:128 * (c + 1)],
                        rhs=qt[:, m0:m0 + 512],
                        start=True,
                        stop=True,
                    )
                pt = ptp.tile([128, 1024], bf16, tag="pt")
                nc.scalar.activation(pt, sps, AF.Exp)
                return pt

            first_av = [True]

            def issue_av(p, g, h, pt):
                for ci in range(2):
                    c = 4 * g + 2 * h + ci
                    for u in range(4):
                        b = 4 * p + u
                        # Exactly one start (global first) and one stop
                        # (global last): a start marks the whole PSUM
                        # zero-region pending-zero for every touched
                        # partition, so a second start would wipe other
                        # blocks' partial sums.  Each block's first touch
                        # still lands as overwrite via pending-zero bytes.
                        nc.tensor.matmul(
                            av[:, 32 * b:32 * (b + 1)],
                            lhsT=pt[:, 512 * ci + 128 * u:512 * ci + 128 * (u + 1)],
                            rhs=vstack[:, D * c:D * (c + 1)],
                            start=first_av[0],
                            stop=(c == NCH - 1 and p == 1 and u == 3),
                            skip_group_check=True,
                        )
                        first_av[0] = False

            def stats_block(b):
                nc.vector.bn_stats(out=stt[:, b, :], in_=av[:, D * b:D * (b + 1)])
                nc.vector.bn_aggr(out=mv[:, b, :], in_=stt[:, b, :])
                nc.vector.tensor_scalar(
                    cent3[:, b, :], av[:, D * b:D * (b + 1)], mv[:, b, 0:1],
                    None, op0=OP.subtract,
                )

            # ---- attention, m-pass outer; AV pipelined one tile behind so
            # the in-order PE never blocks the ACT exp stream ----
            pending = None
            for p in range(2):
                for g in range(NG):
                    if p == 0:
                        prep_group(g)
                    for h in range(2):
                        pt = issue_scores(p, g, h)
                        if pending is not None:
                            issue_av(*pending)
                        pending = (p, g, h, pt)
                if p == 0:
                    # first-half LN stats overlap the second attention pass
                    # (issue the straggler AV first so av cols 0:128 are done)
                    issue_av(*pending)
                    pending = None
                    for b in range(4):
                        stats_block(b)
            issue_av(*pending)

            if debug:
                nc.sync.dma_start(out=dbg_qt[:], in_=qt)
                nc.sync.dma_start(out=dbg_k[:], in_=kstack)
                nc.sync.dma_start(out=dbg_v[:], in_=vstack)
                avst = x2tp.tile([128, NB * D], f32, tag="avdbg")
                nc.vector.tensor_copy(avst, av[:, 0:NB * D])
                nc.sync.dma_start(out=dbg_av[:], in_=avst)

          # ---- phase B tail: stats for the second half, rstd, project ----
          with (
            tc.tile_pool(name="pb_ps", bufs=2, space="PSUM") as pb_ps,
            tc.tile_pool(name="naug", bufs=2) as naugp,
            tc.tile_pool(name="lnagg", bufs=8) as lnagg,
          ):
            magic_f = lnagg.tile([128, NB], f32, name="magicf")
            nc.vector.memset(magic_f, float(np.int32(0x5F3759DF).view(np.float32)))
            magic_i = magic_f.bitcast(mybir.dt.int32)
            for b in range(4, NB):
                stats_block(b)
            # rstd = 1/sqrt(var + eps) via fast-inverse-sqrt + two Newton
            # steps, all in DVE core-ISA ops (mult/subtract/shift) — avoids
            # the ~1.3us ACT Ln-table load on the tail critical path.
            # Signs: each Newton step computes -x_{n+1} from +x_n (the 1.5
            # constant enters negated), so after an even number of steps the
            # result is positive again.
            i32 = mybir.dt.int32
            ve = lnagg.tile([128, NB], f32)
            nc.vector.tensor_scalar_add(ve, mv[:, :, 1], LN_EPS)
            half_i = lnagg.tile([128, NB], i32)
            nc.vector.tensor_scalar(
                half_i, ve.bitcast(i32), 1, None, op0=OP.arith_shift_right
            )
            x0 = lnagg.tile([128, NB], f32)
            nc.vector.tensor_tensor(
                x0.bitcast(i32), magic_i, half_i, op=OP.subtract
            )
            cur = x0
            for it in range(2):
                sq = lnagg.tile([128, NB], f32, name=f"nsq{it}")
                nc.vector.tensor_tensor(sq, cur, cur, op=OP.mult)
                ysq = lnagg.tile([128, NB], f32, name=f"nys{it}")
                nc.vector.tensor_tensor(ysq, ve, sq, op=OP.mult)
                coef = lnagg.tile([128, NB], f32, name=f"ncf{it}")
                nc.vector.tensor_scalar(
                    coef, ysq, 0.5, 1.5, op0=OP.mult, op1=OP.subtract
                )
                nxt = lnagg.tile([128, NB], f32, name=f"nxt{it}")
                nc.vector.tensor_tensor(nxt, cur, coef, op=OP.mult)
                cur = nxt
            rstd = cur
            for b in range(NB):
                nps = pb_ps.tile([D, 128], bf16, tag="pbt")
                nc.tensor.transpose(nps, cent3[:, b, :], identb)
                na = naugp.tile([D, 128], bf16, tag="na")
                # ACT is idle in the tail; GPSIMD cannot read PSUM
                nc.scalar.copy(na, nps)
                ops = pb_ps.tile([128, DOUT], f32, tag="pb")
                nc.tensor.matmul(ops, lhsT=na, rhs=wog_b, start=True, stop=True)
                # out = rstd * (cent @ gWo) + (beta @ Wo + bo)
                nc.vector.scalar_tensor_tensor(
                    out=osb[:, b, :], in0=ops, scalar=rstd[:, b:b + 1],
                    in1=biasb, op0=OP.mult, op1=OP.add,
                )
                if b % 2 == 1:
                    nc.sync.dma_start(
                        out=out_d[(b - 1) * 128:(b + 1) * 128, :].rearrange(
                            "(b p) d -> p b d", p=128
                        ),
                        in_=osb[:, b - 1:b + 1, :],
                    )

    nc.compile()
    _CACHE[key] = nc
    return nc


def host_inputs(input1, input2, Wq, bq, Wk, bk, Wv, bv, gamma, beta, Wo, bo):
    """Per-core input maps (host-side weight folding + constant packing)."""
    f32 = np.float32
    input1 = np.ascontiguousarray(np.asarray(input1, f32))
    input2 = np.ascontiguousarray(np.asarray(input2, f32))
    Wo = np.asarray(Wo, f32)
    try:
        import ml_dtypes
        bf = ml_dtypes.bfloat16
    except ImportError:  # pragma: no cover
        import jax.numpy as jnp
        bf = jnp.bfloat16
    cb = np.zeros((128, CB_W), f32)
    cb[:, CB_ID:CB_ID + 128] = np.eye(128, dtype=f32)
    cb[:, CB_WQ:CB_WQ + D] = np.asarray(Wq, f32)
    cb[:, CB_WK:CB_WK + D] = np.asarray(Wk, f32)
    cb[:, CB_WV:CB_WV + D] = np.asarray(Wv, f32)
    cb[0:D, CB_WOG:CB_WOG + DOUT] = np.asarray(gamma, f32)[:, None] * Wo
    cf = np.zeros((128, CF_W), f32)
    cf[:, CF_BVB:CF_BVB + 128] = np.tile(np.asarray(bv, f32), 4)[None, :]
    cf[:, CF_BIAS:CF_BIAS + DOUT] = (
        np.asarray(beta, f32) @ Wo + np.asarray(bo, f32)
    )[None, :]
    cf[0:D, CF_BQ] = np.asarray(bq, f32)
    cf[0:D, CF_BK] = np.asarray(bk, f32)
    cf[:, CF_EPS] = LN_EPS
    common = {"x2": input2, "cpackb": cb.astype(bf), "cpackf": cf}
    return [
        dict(common, x1=input1[c * MSH:(c + 1) * MSH]) for c in range(NCORES)
    ]


def kernel(input1, input2, Wq, bq, Wk, bk, Wv, bv, gamma, beta, Wo, bo):
    _import_concourse()
    from concourse.bass_utils import run_bass_kernel_spmd

    nc = build()
    in_maps = host_inputs(
        input1, input2, Wq, bq, Wk, bk, Wv, bv, gamma, beta, Wo, bo
    )
    res = run_bass_kernel_spmd(nc, in_maps, list(range(NCORES)))
    return np.concatenate(
        [np.asarray(res.results[c]["out"]) for c in range(NCORES)], axis=0
    ).astype(np.float32)
